# revision 1
# baseline (speedup 1.0000x reference)
"""GATv2Conv Trainium2 kernel — 8-core SPMD, dst-sharded edge parallelism.

Strategy:
  * Sort edges by destination; partition destinations across 8 cores
    (6250 nodes each), so the segment softmax + scatter-add reduce is
    fully core-local (no collectives).
  * Within a core, destinations are processed in 49 blocks of 128 nodes.
    Each block's edge run is padded to a fixed slot count (lo/hi split on
    src < 25088 so gather indices fit in int16).
  * Messages m = W_l x[src] + W_r x[dst] + W_e e are assembled
    feature-major [f, e] on the TensorEngine directly in PSUM:
      - W_e.T @ ea.T            (host-pretransposed edge_attr, bf16)
      - W_l.T @ xgT             (xgT = dma_gather(transpose) of x rows)
      - xr_blk.T @ M01T         (one-hot broadcast of per-dst transform)
  * logits = Mf.T @ attB per 128-edge tile (Mf = LeakyReLU(m), bf16);
    ex = exp(logits) on ScalarE; segment softmax denominator and the
    weighted scatter-add both become matmuls with the one-hot M01 as the
    stationary operand, accumulated in PSUM over the block's edge run.
  * out = num / (den + 1e-16) + bias per dst block, written dense;
    host concatenates core outputs.
"""
import numpy as np
import os
import sys

sys.path.insert(0, "/opt/trn_rl_repo")

import ml_dtypes

BF16NP = ml_dtypes.bfloat16

# ---- problem constants (hardcoded per harness contract) ----
N, E, DIN, H, C, EDIM = 50000, 800000, 128, 4, 32, 64
HC = H * C
NEG = 0.2
NCORES = 8
NPER = N // NCORES


class Cfg:
    def __init__(self, n=N, e=E, ncores=NCORES, xsplit=25088, use_lrelu=True):
        self.n = n
        self.ncores = ncores
        self.nper = n // ncores
        self.nb = (self.nper + 127) // 128          # dst blocks / core
        self.npad = self.nb * 128
        self.xsplit = xsplit                         # lo/hi gather split
        self.xrows = ((n + 127) // 128) * 128        # padded x rows
        self.use_lrelu = use_lrelu
        self.gq = 1 if os.environ.get("GQ", "0") == "1" else 0
        # slot counts filled by host_prep
        self.L = None
        self.Hs = None
        self.EB = None

    @property
    def tb(self):
        return self.EB // 128

    @property
    def cb(self):
        return self.EB // 256

    @property
    def et(self):
        return self.nb * self.EB


def _r128(v):
    return max(128, ((int(v) + 127) // 128) * 128)


def host_prep(cfg, x, edge_attr, W_l, W_r, W_e, att, bias, edge_index):
    """Sort/pad/shard all inputs. Returns (shared, per_core) numpy dicts."""
    src = np.asarray(edge_index[0], np.int64)
    dst = np.asarray(edge_index[1], np.int64)
    x = np.asarray(x, np.float32)
    ea = np.asarray(edge_attr, np.float32)

    order = np.argsort(dst, kind="stable")
    src, dst, ea = src[order], dst[order], ea[order]

    core = dst // cfg.nper
    blk = (dst % cfg.nper) // 128
    ishi = src >= cfg.xsplit

    nb = cfg.nb
    lo_cnt = np.zeros((cfg.ncores, nb), np.int64)
    hi_cnt = np.zeros((cfg.ncores, nb), np.int64)
    gb = core * nb + blk
    np.add.at(lo_cnt.reshape(-1), gb[~ishi], 1)
    np.add.at(hi_cnt.reshape(-1), gb[ishi], 1)

    L = _r128(lo_cnt.max())
    Hs = _r128(hi_cnt.max())
    if ((L + Hs) // 128) % 2:
        Hs += 128
    cfg.L, cfg.Hs, cfg.EB = L, Hs, L + Hs
    EB, ET = cfg.EB, cfg.et

    edim = ea.shape[1]
    per_core = []
    for c in range(cfg.ncores):
        ea_p = np.zeros((ET, edim), np.float32)
        src_p = np.zeros(ET, np.int64)
        dstl_p = np.full(ET, -1.0, np.float32)
        m = core == c
        s_c, d_c, b_c, h_c, ea_c = src[m], dst[m], blk[m], ishi[m], ea[m]
        for b in range(nb):
            mb = b_c == b
            for sel, base in ((~h_c & mb, b * EB), (h_c & mb, b * EB + L)):
                idx = np.nonzero(sel)[0]
                n_ = len(idx)
                sl = slice(base, base + n_)
                ea_p[sl] = ea_c[idx]
                sv = s_c[idx]
                src_p[sl] = np.where(sv >= cfg.xsplit, sv - cfg.xsplit, sv)
                dstl_p[sl] = (d_c[idx] - c * cfg.nper - b * 128).astype(np.float32)
        # pack int16 gather indices: idx i -> [i%16, i//16]; rows 16:32 copy
        idx16 = np.zeros((128, ET // 16), np.int16)
        for b in range(nb):
            for off, cnt in ((b * EB, L), (b * EB + L, Hs)):
                seg = src_p[off:off + cnt].astype(np.int16)
                wr = seg.reshape(cnt // 16, 16).T  # [16, cnt//16]
                idx16[0:16, off // 16:(off + cnt) // 16] = wr
        idx16[16:32] = idx16[0:16]
        # dst_local as [128, ntiles] f32 (tile-major columns)
        dstl_t = dstl_p.reshape(ET // 128, 128).T.copy()
        # x_locT for this core's dst slice [128, npad] bf16
        x_loc = np.zeros((cfg.npad, DIN), np.float32)
        lo = c * cfg.nper
        avail = min(cfg.npad, cfg.n - lo)
        x_loc[:avail] = x[lo:lo + avail]
        per_core.append(dict(
            eaT=np.ascontiguousarray(ea_p.T).astype(BF16NP),
            idx16=idx16,
            dstl=np.ascontiguousarray(dstl_t),
            x_locT=np.ascontiguousarray(x_loc.T).astype(BF16NP),
        ))

    x_pad = np.zeros((cfg.xrows, DIN), np.float32)
    x_pad[:cfg.n] = x
    attB = np.zeros((HC, H), np.float32)
    for h in range(H):
        attB[h * C:(h + 1) * C, h] = np.asarray(att, np.float32)[h]
    if not cfg.use_lrelu:
        # lrelu(x) = 0.4*(1.5x + |x|); device computes Mf = 1.5x + |x|
        attB *= (1.0 - NEG) / 2.0
    shared = dict(
        x_bf=x_pad.astype(BF16NP),
        W_l=np.asarray(W_l, np.float32).astype(BF16NP),
        W_r=np.asarray(W_r, np.float32).astype(BF16NP),
        W_e=np.asarray(W_e, np.float32).astype(BF16NP),
        attB=attB.astype(BF16NP),
        iota=np.tile(np.arange(128, dtype=np.float32), (128, 1)).astype(BF16NP),
        ident=np.eye(128, dtype=np.float32).astype(BF16NP),
        bias_bc=np.tile(np.asarray(bias, np.float32)[None, :], (128, 1)),
    )
    return shared, per_core


def build_program(cfg, ablate=()):
    import concourse.bacc as bacc
    import concourse.bass as bass
    import concourse.tile as tile
    from concourse import mybir

    ab = set(ablate)

    BF = mybir.dt.bfloat16
    F32 = mybir.dt.float32
    I16 = mybir.dt.int16
    AT = mybir.ActivationFunctionType
    OP = mybir.AluOpType

    nb, EB, L, Hs, TB, CB = cfg.nb, cfg.EB, cfg.L, cfg.Hs, cfg.tb, cfg.cb
    ET, NPAD = cfg.et, cfg.npad

    nc = bacc.Bacc("TRN2", target_bir_lowering=False, debug=False)

    x_d = nc.dram_tensor("x_bf", [cfg.xrows, DIN], BF, kind="ExternalInput")
    xlocT_d = nc.dram_tensor("x_locT", [128, NPAD], BF, kind="ExternalInput")
    eaT_d = nc.dram_tensor("eaT", [EDIM, ET], BF, kind="ExternalInput")
    idx_d = nc.dram_tensor("idx16", [128, ET // 16], I16, kind="ExternalInput")
    dstl_d = nc.dram_tensor("dstl", [128, ET // 128], F32, kind="ExternalInput")
    Wl_d = nc.dram_tensor("W_l", [DIN, HC], BF, kind="ExternalInput")
    Wr_d = nc.dram_tensor("W_r", [DIN, HC], BF, kind="ExternalInput")
    We_d = nc.dram_tensor("W_e", [EDIM, HC], BF, kind="ExternalInput")
    attB_d = nc.dram_tensor("attB", [HC, H], BF, kind="ExternalInput")
    iota_d = nc.dram_tensor("iota", [128, 128], BF, kind="ExternalInput")
    ident_d = nc.dram_tensor("ident", [128, 128], BF, kind="ExternalInput")
    bias_d = nc.dram_tensor("bias_bc", [128, HC], F32, kind="ExternalInput")
    out_d = nc.dram_tensor("out", [NPAD, HC], F32, kind="ExternalOutput")

    def bcast_h(ap_2d):
        """[128, H] AP -> [128, H, C] with step-0 inner broadcast."""
        return bass.AP(ap_2d.tensor, ap_2d.offset, [list(ap_2d.ap[0]), [1, H], [0, C]])

    with tile.TileContext(nc) as tc:
        with (
            tc.tile_pool(name="cst", bufs=1) as cst,
            tc.tile_pool(name="gx", bufs=int(os.environ.get("GXB", "2"))) as gx,
            tc.tile_pool(name="eap", bufs=int(os.environ.get("EAB", "2"))) as eap,
            tc.tile_pool(name="mf", bufs=int(os.environ.get("MFB", "5"))) as mfp,
            tc.tile_pool(name="m01", bufs=int(os.environ.get("M01B", "16"))) as m01p,
            tc.tile_pool(name="m01t", bufs=int(os.environ.get("M01TB", "4"))) as m01tp,
            tc.tile_pool(name="zp", bufs=int(os.environ.get("ZB", "8"))) as zp,
            tc.tile_pool(name="exq", bufs=int(os.environ.get("EXB", "8"))) as exq,
            tc.tile_pool(name="ab", bufs=2) as abp,
            tc.tile_pool(name="ep", bufs=2) as epp,
            tc.tile_pool(name="psP", bufs=int(os.environ.get("PSP", "2")), space="PSUM") as psP,
            tc.tile_pool(name="psT", bufs=int(os.environ.get("PST", "1")), space="PSUM") as psT,
            tc.tile_pool(name="psL", bufs=int(os.environ.get("PSL", "2")), space="PSUM") as psL,
            tc.tile_pool(name="psX", bufs=int(os.environ.get("PSX", "2")), space="PSUM") as psX,
            tc.tile_pool(name="psN", bufs=1, space="PSUM") as psN,
        ):
            # ---- constants ----
            Wl_t = cst.tile([DIN, HC], BF, tag="wl")
            nc.sync.dma_start(out=Wl_t[:], in_=Wl_d[:])
            Wr_t = cst.tile([DIN, HC], BF, tag="wr")
            nc.sync.dma_start(out=Wr_t[:], in_=Wr_d[:])
            We_t = cst.tile([EDIM, HC], BF, tag="we")
            nc.sync.dma_start(out=We_t[:], in_=We_d[:])
            attB_t = cst.tile([HC, H], BF, tag="attb")
            nc.sync.dma_start(out=attB_t[:], in_=attB_d[:])
            iota_t = cst.tile([128, 128], BF, tag="iota")
            nc.sync.dma_start(out=iota_t[:], in_=iota_d[:])
            ident_t = cst.tile([128, 128], BF, tag="ident")
            nc.sync.dma_start(out=ident_t[:], in_=ident_d[:])
            bias_t = cst.tile([128, HC], F32, tag="bias")
            nc.sync.dma_start(out=bias_t[:], in_=bias_d[:])
            xlocT_t = cst.tile([128, NPAD], BF, tag="xloct")
            nc.sync.dma_start(out=xlocT_t[:], in_=xlocT_d[:])
            idx_t = cst.tile([128, ET // 16], I16, tag="idx")
            nc.sync.dma_start(out=idx_t[:], in_=idx_d[:])
            dstl_t = cst.tile([128, ET // 128], F32, tag="dstl")
            nc.sync.dma_start(out=dstl_t[:], in_=dstl_d[:])

            # ---- xr_all prologue: per-dst-block target transform ----
            xr_all = cst.tile([128, NPAD], BF, tag="xrall")
            for b in range(nb):
                ps = psX.tile([128, HC], F32, tag="psx")
                nc.tensor.matmul(
                    out=ps[:], lhsT=xlocT_t[:, b * 128:(b + 1) * 128],
                    rhs=Wr_t[:], start=True, stop=True,
                )
                nc.vector.tensor_copy(out=xr_all[:, b * 128:(b + 1) * 128], in_=ps[:])

            # ---- main loop over dst blocks ----
            for b in range(nb):
                xgT = gx.tile([128, EB], BF, tag="xgt")
                icol = b * (EB // 16)
                if "gather" not in ab:
                    nc.gpsimd.dma_gather(
                        out_ap=xgT[:, 0:L].rearrange("p (o e) -> p o e", o=1),
                        in_ap=x_d[0:cfg.xsplit, :],
                        idxs_ap=idx_t[:, icol:icol + L // 16],
                        num_idxs=L, num_idxs_reg=L, elem_size=DIN, transpose=True,
                        single_packet=False, queue_num=0,
                    )
                    nc.gpsimd.dma_gather(
                        out_ap=xgT[:, L:EB].rearrange("p (o e) -> p o e", o=1),
                        in_ap=x_d[cfg.xsplit:cfg.xrows, :],
                        idxs_ap=idx_t[:, icol + L // 16:icol + EB // 16],
                        num_idxs=Hs, num_idxs_reg=Hs, elem_size=DIN, transpose=True,
                        single_packet=False, queue_num=cfg.gq,
                    )
                ea_blk = eap.tile([EDIM, EB], BF, tag="eablk")
                nc.sync.dma_start(out=ea_blk[:], in_=eaT_d[:, b * EB:(b + 1) * EB])

                num_ps = psN.tile([128, HC + H], F32, tag="psn")

                chunks = []
                off = 0
                while off < EB:
                    w = 512 if EB - off >= 512 else EB - off
                    chunks.append((off, w))
                    off += w
                n128 = EB // 128
                t128 = 0
                for (e0, w) in chunks:
                    nt = w // 128
                    P = psP.tile([128, w], F32, tag="psp")
                    if "we" not in ab:
                        nc.tensor.matmul(out=P[:], lhsT=We_t[:], rhs=ea_blk[:, e0:e0 + w],
                                         start=True, stop=False)
                    if "wl" not in ab:
                        nc.tensor.matmul(out=P[:], lhsT=Wl_t[:], rhs=xgT[:, e0:e0 + w],
                                         start=False, stop=False)
                    m01T_ps = psT.tile([128, w], BF, tag="pst")
                    m01s = []
                    for t in range(nt):
                        gt = b * TB + t128 + t
                        m01 = m01p.tile([128, 128], BF, tag="m01")
                        if "m01" not in ab:
                            nc.vector.tensor_scalar(
                                out=m01[:], in0=iota_t[:],
                                scalar1=dstl_t[:, gt:gt + 1], scalar2=None,
                                op0=OP.is_equal,
                            )
                        if "m01t" not in ab:
                            nc.tensor.transpose(
                                out=m01T_ps[:, t * 128:(t + 1) * 128], in_=m01[:],
                                identity=ident_t[:],
                            )
                        m01s.append(m01)
                    m01T = m01tp.tile([128, w], BF, tag="m01t")
                    if "m01t" not in ab:
                        if os.environ.get("CPYV", "1") == "1":
                            nc.vector.tensor_copy(out=m01T[:], in_=m01T_ps[:])
                        else:
                            nc.any.tensor_copy(out=m01T[:], in_=m01T_ps[:])
                    if "xr" not in ab:
                        nc.tensor.matmul(out=P[:], lhsT=xr_all[:, b * 128:(b + 1) * 128],
                                         rhs=m01T[:], start=False, stop=True)

                    Mf = mfp.tile([128, w], BF, tag="mf")
                    if "lrelu" in ab:
                        pass
                    elif cfg.use_lrelu:
                        nc.scalar.activation(out=Mf[:], in_=P[:], func=AT.Prelu,
                                             alpha=NEG)
                    else:
                        Ab = abp.tile([128, w], BF, tag="ab")
                        nc.scalar.activation(out=Ab[:], in_=P[:], func=AT.Abs)
                        nc.vector.scalar_tensor_tensor(
                            out=Mf[:], in0=P[:], scalar=(1.0 + NEG) / (1.0 - NEG),
                            in1=Ab[:], op0=OP.mult, op1=OP.add,
                        )
                    lg_ps = psL.tile([128, 4 * H], F32, tag="psl")
                    for t in range(nt):
                        Z = zp.tile([128, HC + H], BF, tag="z")
                        if "logits" not in ab:
                            nc.tensor.matmul(out=lg_ps[:, t * H:(t + 1) * H],
                                             lhsT=Mf[:, t * 128:(t + 1) * 128],
                                             rhs=attB_t[:], start=True, stop=True)
                        xle = psX.tile([128, HC], F32, tag="psx")
                        if "xle" not in ab:
                            nc.tensor.matmul(out=xle[:],
                                             lhsT=xgT[:, e0 + t * 128:e0 + (t + 1) * 128],
                                             rhs=Wl_t[:], start=True, stop=True)
                        if "exp" not in ab:
                            nc.scalar.activation(out=Z[:, HC:HC + H],
                                                 in_=lg_ps[:, t * H:(t + 1) * H],
                                                 func=AT.Exp)
                        if "z" not in ab:
                            nc.vector.tensor_tensor(
                                out=Z[:, 0:HC].rearrange("p (h c) -> p h c", h=H),
                                in0=xle[:].rearrange("p (h c) -> p h c", h=H),
                                in1=bcast_h(Z[:, HC:HC + H]),
                                op=OP.mult,
                            )
                        first = (e0 == 0 and t == 0)
                        last = (t128 + t == n128 - 1)
                        if "numden" not in ab:
                            nc.tensor.matmul(out=num_ps[:], lhsT=m01s[t][:],
                                             rhs=Z[:], start=first, stop=last)
                    t128 += nt

                # ---- block epilogue: divide + bias + store ----
                den = epp.tile([128, H], F32, tag="den")
                nc.vector.tensor_scalar(out=den[:], in0=num_ps[:, HC:HC + H],
                                        scalar1=1e-16, scalar2=None, op0=OP.add)
                rden = epp.tile([128, H], F32, tag="rden")
                nc.vector.reciprocal(out=rden[:], in_=den[:])
                osb = epp.tile([128, HC], F32, tag="osb")
                nc.vector.tensor_tensor(
                    out=osb[:].rearrange("p (h c) -> p h c", h=H),
                    in0=num_ps[:, 0:HC].rearrange("p (h c) -> p h c", h=H),
                    in1=bcast_h(rden[:]),
                    op=OP.mult,
                )
                nc.vector.tensor_add(out=osb[:], in0=osb[:], in1=bias_t[:])
                nc.sync.dma_start(out=out_d[b * 128:(b + 1) * 128, :], in_=osb[:])

    nc.compile()
    return nc


def make_in_maps(cfg, shared, per_core):
    maps = []
    for c in range(cfg.ncores):
        pc = per_core[c]
        maps.append({
            "x_bf": shared["x_bf"],
            "x_locT": pc["x_locT"],
            "eaT": pc["eaT"],
            "idx16": pc["idx16"],
            "dstl": pc["dstl"],
            "W_l": shared["W_l"],
            "W_r": shared["W_r"],
            "W_e": shared["W_e"],
            "attB": shared["attB"],
            "iota": shared["iota"],
            "ident": shared["ident"],
            "bias_bc": shared["bias_bc"],
        })
    return maps


_cache = {}


def kernel(**inputs):
    from concourse.bass_utils import run_bass_kernel_spmd

    import os
    cfg = Cfg(use_lrelu=os.environ.get("LRELU", "1") == "1")
    shared, per_core = host_prep(cfg, **inputs)
    key = (cfg.L, cfg.Hs)
    if key not in _cache:
        _cache[key] = build_program(cfg)
    nc = _cache[key]
    in_maps = make_in_maps(cfg, shared, per_core)
    res = run_bass_kernel_spmd(nc, in_maps, core_ids=list(range(cfg.ncores)))
    out = np.zeros((N, HC), np.float32)
    for c in range(cfg.ncores):
        out[c * NPER:(c + 1) * NPER] = res.results[c]["out"][:NPER]
    return out

